# revision 26
# baseline (speedup 1.0000x reference)
"""Trainium2 Bass kernel for nn_CompositionalNetwork (ragged_sequence).

Computation: per-token embedding concat (word[200] ++ tag[20]) followed by a
per-chunk-length Linear (chunks of 1..4 consecutive tokens), scattered to the
output row given by pos.

v4: fp8 E3M4 operand slabs + partition-major DRAM packing + full-128
matmul tiles. The host packs each length-group's full 128-row contraction
segments as [128, nseg-1, 5120] (SBUF layout) so every SBUF partition's
per-k payload is one contiguous DRAM run (5-30 KB descriptors); the partial
tail segments ship unpadded ([rs, 5120], 5 KB lines) which saves the 9% of
traffic the pad rows would cost. Chunk tiles are 128 wide (5120 slots, 120
pads whose output rows land >= 5000 and are ignored) so the PE's Fast
Weight Load engages (NumWeights==128) and the fp8 LDWEIGHTS (~27 ns) hides
behind the 83 ns moving stream -- M=125 tiles measured 125 us vs 100 us
for M=128.

Numerics: embeddings pre-scaled by 64 and cast to E3M4 (4 mantissa bits);
the 1/64 is folded into the bf16 weight slab. Measured end-to-end rel err
1.235e-2 vs the 2e-2 gate (e4m3 at 2.3e-2 fails, which is why E3M4). Mixed
fp8xbf16 matmul upconverts to FP22 in the PE -- verified on HW.

Device kernel per core:
  - 9 load DMAs (12.2 MB) cycling SP/ACT/Pool rings, 760 matmuls (lhsT =
    fp8e3 chunk columns, rhs = bf16 packed W rows, free dim 200, f32 PSUM),
    DVE drains PSUM pairs to a bf16 staging tile, 8 store DMAs (8 KB
    lines, 8.2 MB) alternating SP/ACT.
  - Measured per-core body on HW (x64 in-NEFF amplification, all 8 cores
    streaming): ~100 us full, ~95 us DMA-only -> the wall is the ~215 GB/s
    effective per-core HBM stream; PE/DVE are hidden behind it.

Sharding: data-parallel over chunks, core c takes chunks [c*5000,(c+1)*5000)
of every length group; the host applies the pos scatter and upcasts to f32.
"""
import numpy as np
import ml_dtypes

bf16 = ml_dtypes.bfloat16
f8e3 = ml_dtypes.float8_e3m4

VOCAB = 128000
TAGS = 64
WD = 200
TD = 20
E = WD + TD       # 220
CD = 200
K = 4
C = 40000
S = 400000
NCH = K * C

NCORES = 8
P = 128
CPG = C // NCORES          # real chunks per group per core (5000)
NT = 40                    # tiles per group per core
M = 128                    # chunk columns per tile: full 128 so the PE's
                           # Fast Weight Load engages (4 fp8 cols per 32-bit
                           # read needs NumWeights==128); slots with p>=125
                           # are pads whose output rows land >= 5000 and are
                           # simply ignored on unshard (+2.4% bytes)
CG = NT * M                # padded chunk slots per group per core (5120)
CR = {k: E * k + 1 for k in range(1, K + 1)}        # contraction rows
NSEG = {k: -(-CR[k] // P) for k in range(1, K + 1)}  # 2,4,6,7
SEGBASE = {1: 0, 2: 2, 3: 6, 4: 12}
NSEGTOT = 19
XSCALE = 64.0              # fp8 pre-scale on x; 1/64 folded into W on host

_CACHE = {}


def _build_kernel(loops=1, probe=None):
    # probe (bench-only ablations, graded path uses None):
    #   "nope":  DMAs only (no matmuls, no drains) -> isolates DMA wall
    #   "nodve": DMAs + matmuls (no PSUM drains)   -> adds PE serial cost
    from concourse import bacc
    import concourse.tile as tile
    from concourse import mybir
    import concourse.bass as bass

    nc = bacc.Bacc(None)

    # full 128-row segments, partition-major (one contiguous DRAM run per
    # partition line)
    xt_d = {
        k: nc.dram_tensor(f"xt{k}", [P, NSEG[k] - 1, CG], mybir.dt.float8e3,
                          kind="ExternalInput")
        for k in range(1, K + 1)
    }
    # partial tail segments, packed without the zero-pad rows (saves 9% of
    # input bytes; 5 KB lines instead of pad traffic)
    xtt_d = {
        k: nc.dram_tensor(f"xtt{k}", [CR[k] - (NSEG[k] - 1) * P, CG],
                          mybir.dt.float8e3, kind="ExternalInput")
        for k in range(1, K + 1)
    }
    # host-packed in SBUF layout: partition p holds all 19 seg rows
    wsb_d = nc.dram_tensor("wsb", [P, NSEGTOT * CD], mybir.dt.bfloat16,
                           kind="ExternalInput")
    out = nc.dram_tensor("out", [K, CG, CD], mybir.dt.bfloat16,
                         kind="ExternalOutput")

    with tile.TileContext(nc) as tc:
        with (
            tc.tile_pool(name="singles", bufs=1) as singles,
            tc.tile_pool(name="ysp", bufs=4) as ysp,
            tc.tile_pool(name="ypp", bufs=8, space="PSUM") as ypp,
        ):
            # phase-separated traffic: loads alternate SP (HWDGE) and Pool
            # (SWDGE) rings; ALL stores go on the ACT ring behind a dummy
            # DMA that waits for the last loads, so reads stream the HBM
            # stack without write-turnaround mixing, and the deferred
            # writes burst behind the (slower) PE pipeline
            ldq = [nc.sync, nc.gpsimd]
            lq = [0]

            def dma(out_ap, in_ap):
                if out_ap.space.name == "SBUF":
                    ldq[lq[0] & 1].dma_start(out=out_ap, in_=in_ap)
                    lq[0] += 1
                else:
                    nc.scalar.dma_start(out=out_ap, in_=in_ap)

            wsb = singles.tile([P, NSEGTOT, CD], mybir.dt.bfloat16)
            # landing pad for the store-deferral dummy DMA
            dpad = singles.tile([1, NSEG[K], 16], mybir.dt.float8e3)
            pstage = None
            if probe is not None:
                # probe stores stream from a once-initialized dummy tile so
                # they carry no compute dependency (pure-DMA ablation)
                pstage = singles.tile([M, NT, CD], mybir.dt.bfloat16,
                                      name="pstage")
                nc.vector.memset(pstage[:], 0.0)
            xts = {
                k: singles.tile([P, NSEG[k], CG], mybir.dt.float8e3,
                                name=f"xts{k}")
                for k in range(1, K + 1)
            }

            for lp in range(loops):
                # all loads up front: weights, then per-k full-seg main +
                # unpadded partial tail
                dma(wsb[:], wsb_d[:])
                for k in range(1, K + 1):
                    nseg = NSEG[k]
                    nfull = nseg - 1
                    dma(xts[k][:, 0:nfull, :], xt_d[k][:])
                    rs = CR[k] - nfull * P
                    dma(xts[k][0:rs, nfull, :], xtt_d[k][:])
                # dummy on the store (ACT) ring reading a sliver that spans
                # both k4 loads: per-ring FIFO then means every load has
                # landed before the first store descriptor enqueues
                nc.scalar.dma_start(out=dpad[:], in_=xts[K][0:1, :, 0:16])

                for k in range(1, K + 1):
                    nseg = NSEG[k]
                    if probe is None:
                        ystage = ysp.tile([M, NT, CD], mybir.dt.bfloat16,
                                          name="ystage")
                    else:
                        ystage = pstage
                    for tp in range(NT // 2):
                        # two column-tiles share one PSUM bank; one DVE drain
                        # per pair halves the fixed PSUM-access cost
                        if probe == "nope":
                            continue
                        y = ypp.tile([M, 2, CD], mybir.dt.float32)
                        for tt in range(2):
                            t = 2 * tp + tt
                            for s in range(nseg):
                                rs = min(P, CR[k] - s * P)
                                nc.tensor.matmul(
                                    y[:, tt, :],
                                    lhsT=xts[k][0:rs, s, t * M:(t + 1) * M],
                                    rhs=wsb[0:rs, SEGBASE[k] + s, :],
                                    start=(s == 0), stop=(s == nseg - 1),
                                )
                        if probe == "nodve":
                            continue
                        # partitions >= 125 are pad slots (out rows >= 5000)
                        nc.vector.tensor_copy(
                            ystage[0:125, 2 * tp:2 * tp + 2, :], y[0:125])
                    # local out row = partition*NT + tile -> contiguous per
                    # partition line; half DMAs (8 KB lines) so draining
                    # starts mid-k
                    nq = NT // 2
                    for h in range(2):
                        dst = bass.AP(
                            tensor=out[:].tensor,
                            offset=(k - 1) * CG * CD + h * nq * CD,
                            ap=[[NT * CD, 125], [CD, nq], [1, CD]],
                        )
                        dma(dst, ystage[0:125, h * nq:(h + 1) * nq, :])
    nc.compile()
    return nc


def _prep(inputs):
    """Host-side shard + pack. Returns in_maps (one dict per core)."""
    tok = np.asarray(inputs["token_indices"]).astype(np.int64)
    tagi = np.asarray(inputs["tag_indices"]).astype(np.int64)
    word_q = (np.asarray(inputs["word_table"], dtype=np.float32)
              .astype(bf16).astype(np.float32) * XSCALE).astype(f8e3)
    tag_q = (np.asarray(inputs["tag_table"], dtype=np.float32)
             .astype(bf16).astype(np.float32) * XSCALE).astype(f8e3)

    # packed weights: rows of [W_k.T/XSCALE ; b_k] split into 128-row
    # segments, stored pre-transposed in the SBUF layout [partition, seg, CD]
    wsb = np.zeros((P, NSEGTOT, CD), dtype=np.float32)
    for k in range(1, K + 1):
        Wk = np.asarray(inputs[f"W{k}"], dtype=np.float32)
        bk = np.asarray(inputs[f"b{k}"], dtype=np.float32)
        Wa = np.concatenate([Wk.T / XSCALE, bk[None, :]], axis=0)
        for s in range(NSEG[k]):
            rs = min(P, CR[k] - s * P)
            wsb[0:rs, SEGBASE[k] + s] = Wa[s * P:s * P + rs]
    wsb = wsb.reshape(P, NSEGTOT * CD).astype(bf16)

    # column c of xt holds chunk slot (tile t = c//M, partition p = c%M)
    # whose local output row is r = p*NT + t (contiguous per-partition out);
    # slots with r >= CPG are pads (junk in, ignored out)
    cols = np.arange(CG)
    rloc = (cols % M) * NT + cols // M
    rsafe = np.minimum(rloc, CPG - 1)
    one = np.float32(1.0).astype(f8e3)

    in_maps = []
    for c in range(NCORES):
        base = c * CPG
        m = {"wsb": wsb}
        for k in range(1, K + 1):
            starts = np.asarray(inputs[f"starts{k}"]).astype(np.int64)
            st = starts[base + rsafe]
            nseg = NSEG[k]
            nfull = nseg - 1
            X = np.zeros((nseg * P, CG), dtype=f8e3)
            for j in range(k):
                tj = np.clip(st + j, 0, S - 1)
                X[j * E:j * E + WD] = word_q[tok[tj]].T
                X[j * E + WD:(j + 1) * E] = tag_q[tagi[tj]].T
            X[E * k] = one
            # main: partition-major [128, nfull, CG]; tail: unpadded rows
            m[f"xt{k}"] = np.ascontiguousarray(
                X[0:nfull * P].reshape(nfull, P, CG).transpose(1, 0, 2))
            m[f"xtt{k}"] = np.ascontiguousarray(X[nfull * P:CR[k]])
        in_maps.append(m)
    return in_maps


def kernel(**inputs) -> np.ndarray:
    from concourse.bass_utils import run_bass_kernel_spmd

    in_maps = _prep(inputs)

    if "nc" not in _CACHE:
        _CACHE["nc"] = _build_kernel()
    nc = _CACHE["nc"]

    res = run_bass_kernel_spmd(nc, in_maps, list(range(NCORES)))

    out_full = np.zeros((NCH, CD), dtype=np.float32)
    for c in range(NCORES):
        o = np.asarray(res.results[c]["out"]).astype(np.float32)
        base = c * CPG
        for k in range(1, K + 1):
            pos = np.asarray(inputs[f"pos{k}"]).astype(np.int64)
            out_full[pos[base:base + CPG]] = o[k - 1, :CPG]
    return out_full
